# revision 5
# baseline (speedup 1.0000x reference)
"""BitNet linear kernel for 8x Trainium2 NeuronCores.

Computes: alpha = mean(|W|); W_q = sign(W) * (|W| > alpha/2) * alpha
          out  = sign(x) @ W_q^T + bias         (x: [4,2048,4096] f32,
                                                 W: [16384,4096] f32)

Sharding: column-parallel (tensor-parallel over out_features).  Each of the
8 cores receives the full x and a 2048-row shard of W; alpha's global
|W|-sum is all-reduced on-device across the shards.  Host code only
slices inputs and concatenates output shards.

Device algorithm per core:
  1. partial = sum|W_shard| (DVE abs-reduce), cross-partition reduce +
     broadcast via ones-matmul, AllReduce(add) across 8 cores.
     th = alpha/2 = total * 0.5/(16384*4096)   (exact pow2 scale)
  2. W ternary quantize: t2 = Sign(w - th) + Sign(w + th)  in {-2,0,2},
     bf16, written to DRAM, re-read with DMA-transpose into a persistent
     SBUF tile wqT[128, K/128, 2048]  (k = ksub*128 + partition).
  3. x quantize: xq = Sign(x) bf16, DRAM round-trip, DMA-transpose to
     xqT[128, K/128, MB] per m-block.
  4. matmul: psum[m,n] += xqT^T @ wqT (PE, bf16), bias folded in via a
     rank-1 ones x (bias*2/alpha) matmul, eviction on ACT with
     scale=alpha/2.  out = (2k + 2*bias/alpha) * alpha/2 = k*alpha + bias.
"""

import os
import sys

import numpy as np

if "/opt/trn_rl_repo" not in sys.path:
    sys.path.insert(0, "/opt/trn_rl_repo")

import concourse.bacc as bacc
import concourse.bass as bass
import concourse.mybir as mybir
import concourse.tile as tile
from concourse.bass import ds, ts
from concourse.bass_utils import run_bass_kernel_spmd

F32 = mybir.dt.float32
BF16 = mybir.dt.bfloat16
P = 128

N_CORES = 8
B, S, DIN, DOUT = 4, 2048, 4096, 16384


def build_nc(
    M=B * S,
    K=DIN,
    N=DOUT // N_CORES,
    dout_total=DOUT,
    n_cores=N_CORES,
    MB=256,
    debug=False,
):
    """Build the per-core Bass program (SPMD: same NEFF on all cores)."""
    KSUB = K // P  # k-subtiles
    NWT = N // P  # W row-tiles per shard
    N_FREE = min(512, N)  # psum free width
    NT = N // N_FREE  # n-chunks
    MT = MB // P  # m-tiles per m-block
    M_BLOCKS = M // MB
    XCH = min(2048, K)  # f32 staging chunk
    NCH = K // XCH
    half_scale = 0.5 / (dout_total * K)  # alpha/2 = total * half_scale

    nc = bacc.Bacc(
        "TRN2",
        target_bir_lowering=False,
        debug=debug,
        num_devices=n_cores,
    )

    x_in = nc.dram_tensor("x", [M, K], F32, kind="ExternalInput")
    w_in = nc.dram_tensor("w", [N, K], F32, kind="ExternalInput")
    b_in = nc.dram_tensor("b", [N], F32, kind="ExternalInput")
    out_d = nc.dram_tensor("out", [M, N], F32, kind="ExternalOutput")

    wq_dram = nc.dram_tensor("wq_dram", [N, K], BF16)
    cc_in = nc.dram_tensor("cc_in", [1, 1], F32)
    cc_out = nc.dram_tensor("cc_out", [1, 1], F32, addr_space="Shared")

    with tile.TileContext(nc) as tc:
        with (
            tc.tile_pool(name="const", bufs=1) as constp,
            tc.tile_pool(name="wqt", bufs=1) as wqtp,
            tc.tile_pool(name="dram", bufs=2, space="DRAM") as dramp,
        ):
            # ---------- constants ----------
            ones_f32 = constp.tile([P, P], F32)
            nc.vector.memset(ones_f32, 1.0)
            ones_row = constp.tile([1, P], BF16)
            nc.vector.memset(ones_row, 1.0)

            # ---------- stage A: alpha ----------
            wacc = constp.tile([P, NWT * NCH], F32)
            with tc.tile_pool(name="wload", bufs=3) as wload:
                for t in range(NWT):
                    for h in range(NCH):
                        wt = wload.tile([P, XCH], F32, tag="wt")
                        nc.sync.dma_start(wt, w_in[ts(t, P), ts(h, XCH)])
                        nc.vector.tensor_reduce(
                            wacc[:, t * NCH + h : t * NCH + h + 1],
                            wt,
                            axis=mybir.AxisListType.X,
                            op=mybir.AluOpType.add,
                            apply_absolute_value=True,
                        )
            wsum = constp.tile([P, 1], F32)
            nc.vector.tensor_reduce(
                wsum, wacc, axis=mybir.AxisListType.X, op=mybir.AluOpType.add
            )
            with tc.tile_pool(name="pss", bufs=1, space="PSUM") as pss:
                # ones^T @ wsum : cross-partition reduce, broadcast to all 128
                shard_ps = pss.tile([P, 1], F32)
                nc.tensor.matmul(shard_ps, ones_f32, wsum, start=True, stop=True)
                shard_tot = constp.tile([P, 1], F32)
                nc.scalar.copy(shard_tot, shard_ps)

            nc.sync.dma_start(cc_in[:, :], shard_tot[0:1, :])
            nc.gpsimd.collective_compute(
                "AllReduce",
                mybir.AluOpType.add,
                replica_groups=[list(range(n_cores))],
                ins=[cc_in[:, :].opt()],
                outs=[cc_out[:, :].opt()],
            )
            tot_sb = constp.tile([1, 1], F32)
            nc.sync.dma_start(tot_sb, cc_out[:, :])

            th_pos = constp.tile([P, 1], F32)  # +alpha/2 (also out scale)
            th_neg = constp.tile([P, 1], F32)  # -alpha/2
            with tc.tile_pool(name="pss2", bufs=1, space="PSUM") as pss2:
                tot_ps = pss2.tile([P, 1], F32)
                nc.tensor.matmul(
                    tot_ps, ones_f32[0:1, :], tot_sb, start=True, stop=True
                )
                nc.scalar.mul(th_pos, tot_ps, half_scale)
                nc.scalar.mul(th_neg, tot_ps, -half_scale)

            # bias row scaled by 2/alpha (rank-1 matmul feeds psum with
            # bias * 2/alpha, eviction scale alpha/2 restores bias)
            inv_th = constp.tile([1, 1], F32)
            nc.vector.reciprocal(inv_th, th_pos[0:1, :])
            bias2 = constp.tile([1, N], BF16)
            with tc.tile_pool(name="btmp", bufs=1) as btmp:
                brow = btmp.tile([1, N], F32)
                nc.sync.dma_start(brow, b_in[:])
                nc.vector.tensor_scalar(
                    bias2, brow, inv_th[0:1, 0:1], None, mybir.AluOpType.mult
                )

            # ---------- stage B: quantize W + transpose ----------
            with (
                tc.tile_pool(name="wload2", bufs=3) as wload2,
                tc.tile_pool(name="wsign", bufs=2) as wsign,
            ):
                for t in range(NWT):
                    for h in range(NCH):
                        wt = wload2.tile([P, XCH], F32, tag="wt2")
                        nc.sync.dma_start(wt, w_in[ts(t, P), ts(h, XCH)])
                        sp = wsign.tile([P, XCH], BF16, tag="sp")
                        sm = wsign.tile([P, XCH], BF16, tag="sm")
                        nc.scalar.activation(
                            sp, wt, mybir.ActivationFunctionType.Sign,
                            bias=th_neg[:, 0:1],
                        )
                        nc.scalar.activation(
                            sm, wt, mybir.ActivationFunctionType.Sign,
                            bias=th_pos[:, 0:1],
                        )
                        wq = wsign.tile([P, XCH], BF16, tag="wq")
                        nc.vector.tensor_tensor(wq, sp, sm, mybir.AluOpType.add)
                        nc.sync.dma_start(wq_dram[ts(t, P), ts(h, XCH)], wq)

            wqT = wqtp.tile([P, KSUB, N], BF16)  # persistent, k=ks*128+p
            for ks in range(KSUB):
                nc.sync.dma_start(
                    wqT[:, ks, :], wq_dram[:, ts(ks, P)], transpose=True
                )

            # ---------- stage C/D: main loop over m-blocks ----------
            with (
                tc.tile_pool(name="xload", bufs=2) as xload,
                tc.tile_pool(name="xsign", bufs=2) as xsign,
                tc.tile_pool(name="xqt", bufs=2) as xqtp,
                tc.tile_pool(name="psum", bufs=2, space="PSUM") as psp,
                tc.tile_pool(name="oev", bufs=1) as oev,
            ):
                for mb in range(M_BLOCKS):
                    xq_d = dramp.tile([MB, K], BF16, tag="xq_d")
                    for mi in range(MT):
                        row0 = mb * MB + mi * P
                        for h in range(NCH):
                            xt = xload.tile([P, XCH], F32, tag="xt")
                            nc.sync.dma_start(
                                xt, x_in[ds(row0, P), ts(h, XCH)]
                            )
                            xq = xsign.tile([P, XCH], BF16, tag="xq")
                            nc.scalar.activation(
                                xq, xt, mybir.ActivationFunctionType.Sign
                            )
                            nc.sync.dma_start(
                                xq_d[ds(mi * P, P), ts(h, XCH)], xq
                            )
                    xqT = xqtp.tile([P, KSUB, MB], BF16, tag="xqT")
                    for ks in range(KSUB):
                        nc.sync.dma_start(
                            xqT[:, ks, :], xq_d[:, ts(ks, P)], transpose=True
                        )
                    for mi in range(MT):
                        row0 = mb * MB + mi * P
                        pst = [
                            psp.tile(
                                [P, N_FREE], F32, tag=f"ps{n}", name=f"ps{n}"
                            )
                            for n in range(NT)
                        ]
                        for n in range(NT):
                            nc.tensor.matmul(
                                pst[n],
                                ones_row,
                                bias2[:, ts(n, N_FREE)],
                                start=True,
                                stop=False,
                            )
                        for ks in range(KSUB):
                            lhs = xqT[:, ks, ds(mi * P, P)]
                            for n in range(NT):
                                nc.tensor.matmul(
                                    pst[n],
                                    lhs,
                                    wqT[:, ks, ts(n, N_FREE)],
                                    start=False,
                                    stop=(ks == KSUB - 1),
                                )
                        for n in range(NT):
                            ot = oev.tile([P, N_FREE], F32, tag=f"ot{n}")
                            nc.scalar.activation(
                                ot,
                                pst[n],
                                mybir.ActivationFunctionType.Copy,
                                bias=0.0,
                                scale=th_pos[:, 0:1],
                            )
                            nc.sync.dma_start(
                                out_d[ds(row0, P), ts(n, N_FREE)], ot
                            )

    nc.compile()
    return nc


def build_nc_v2(
    M=B * S,
    K=DIN,
    N=DOUT // N_CORES,
    dout_total=DOUT,
    n_cores=N_CORES,
    debug=False,
):
    """V2: fp8e4 DoubleRow matmul (2x PE), SBUF->SBUF DMA-transposes (no
    DRAM round-trips), eviction + exact bias add fused on DVE."""
    FP8 = mybir.dt.float8e4
    KSUB = K // P
    assert KSUB % 2 == 0, "DoubleRow needs even k-subtile count"
    NWT = N // P
    N_FREE = min(512, N)
    NT = N // N_FREE
    MTILES = M // P
    XCH = min(2048, K)
    NCH = K // XCH
    half_scale = 0.5 / (dout_total * K)

    nc = bacc.Bacc(
        "TRN2",
        target_bir_lowering=False,
        debug=debug,
        num_devices=n_cores,
    )

    x_in = nc.dram_tensor("x", [M, K], F32, kind="ExternalInput")
    w_in = nc.dram_tensor("w", [N, K], F32, kind="ExternalInput")
    b_in = nc.dram_tensor("b", [N], F32, kind="ExternalInput")
    out_d = nc.dram_tensor("out", [M, N], F32, kind="ExternalOutput")
    cc_in = nc.dram_tensor("cc_in", [1, 1], F32)
    cc_out = nc.dram_tensor("cc_out", [1, 1], F32, addr_space="Shared")

    with tile.TileContext(nc) as tc:
        with (
            tc.tile_pool(name="const", bufs=1) as constp,
            tc.tile_pool(name="wqt", bufs=1) as wqtp,
        ):
            ones_f32 = constp.tile([P, P], F32)
            nc.vector.memset(ones_f32, 1.0)

            # ---------- stage A: alpha ----------
            wacc = constp.tile([P, NWT * NCH], F32)
            with tc.tile_pool(name="wload", bufs=3) as wload:
                for t in range(NWT):
                    for h in range(NCH):
                        wt = wload.tile([P, XCH], F32, tag="wt")
                        nc.sync.dma_start(wt, w_in[ts(t, P), ts(h, XCH)])
                        nc.vector.tensor_reduce(
                            wacc[:, t * NCH + h : t * NCH + h + 1],
                            wt,
                            axis=mybir.AxisListType.X,
                            op=mybir.AluOpType.add,
                            apply_absolute_value=True,
                        )
            wsum = constp.tile([P, 1], F32)
            nc.vector.tensor_reduce(
                wsum, wacc, axis=mybir.AxisListType.X, op=mybir.AluOpType.add
            )
            with tc.tile_pool(name="pss", bufs=1, space="PSUM") as pss:
                shard_ps = pss.tile([P, 1], F32)
                nc.tensor.matmul(shard_ps, ones_f32, wsum, start=True, stop=True)
                shard_tot = constp.tile([P, 1], F32)
                nc.scalar.copy(shard_tot, shard_ps)

            nc.sync.dma_start(cc_in[:, :], shard_tot[0:1, :])
            nc.gpsimd.collective_compute(
                "AllReduce",
                mybir.AluOpType.add,
                replica_groups=[list(range(n_cores))],
                ins=[cc_in[:, :].opt()],
                outs=[cc_out[:, :].opt()],
            )
            tot_sb = constp.tile([1, 1], F32)
            nc.sync.dma_start(tot_sb, cc_out[:, :])

            th_pos = constp.tile([P, 1], F32)
            th_neg = constp.tile([P, 1], F32)
            with tc.tile_pool(name="pss2", bufs=1, space="PSUM") as pss2:
                tot_ps = pss2.tile([P, 1], F32)
                nc.tensor.matmul(
                    tot_ps, ones_f32[0:1, :], tot_sb, start=True, stop=True
                )
                nc.scalar.mul(th_pos, tot_ps, half_scale)
                nc.scalar.mul(th_neg, tot_ps, -half_scale)

            # exact f32 bias broadcast to all partitions via fp32 rank-1
            bias_bc = constp.tile([P, N], F32)
            with (
                tc.tile_pool(name="btmp", bufs=1) as btmp,
                tc.tile_pool(name="bps", bufs=2, space="PSUM") as bps,
            ):
                brow = btmp.tile([1, N], F32)
                nc.sync.dma_start(brow, b_in[:])
                for n in range(NT):
                    bp = bps.tile([P, N_FREE], F32, tag="bp", name="bp")
                    nc.tensor.matmul(
                        bp,
                        ones_f32[0:1, :],
                        brow[:, ts(n, N_FREE)],
                        start=True,
                        stop=True,
                    )
                    nc.vector.tensor_copy(bias_bc[:, ts(n, N_FREE)], bp)

            # ---------- stage B: quantize W + transpose (SBUF->SBUF) ----------
            wqT8 = wqtp.tile([P, KSUB, N], FP8)  # persistent, k=ks*128+p
            with (
                tc.tile_pool(name="wload2", bufs=3) as wload2,
                tc.tile_pool(name="wsign", bufs=2) as wsign,
                tc.tile_pool(name="wtr", bufs=2) as wtr,
            ):
                for t in range(NWT):
                    wq = wsign.tile([P, K], BF16, tag="wq")
                    for h in range(NCH):
                        wt = wload2.tile([P, XCH], F32, tag="wt2")
                        nc.sync.dma_start(wt, w_in[ts(t, P), ts(h, XCH)])
                        sp = wsign.tile([P, XCH], BF16, tag="sp")
                        sm = wsign.tile([P, XCH], BF16, tag="sm")
                        nc.scalar.activation(
                            sp, wt, mybir.ActivationFunctionType.Sign,
                            bias=th_neg[:, 0:1],
                        )
                        nc.scalar.activation(
                            sm, wt, mybir.ActivationFunctionType.Sign,
                            bias=th_pos[:, 0:1],
                        )
                        nc.vector.tensor_tensor(
                            wq[:, ts(h, XCH)], sp, sm, mybir.AluOpType.add
                        )
                    wqTb = wtr.tile([P, KSUB, P], BF16, tag="wqTb")
                    for ks in range(KSUB):
                        nc.sync.dma_start(
                            wqTb[:, ks, :], wq[:, ts(ks, P)], transpose=True
                        )
                    nc.vector.tensor_copy(wqT8[:, :, ts(t, P)], wqTb)

            # ---------- stage C: main loop ----------
            with (
                tc.tile_pool(name="xload", bufs=4) as xload,
                tc.tile_pool(name="xsign", bufs=2) as xsign,
                tc.tile_pool(name="xtr", bufs=2) as xtr,
                tc.tile_pool(name="xq8", bufs=2) as xq8p,
                tc.tile_pool(name="psum", bufs=2, space="PSUM") as psp,
                tc.tile_pool(name="oev", bufs=2) as oev,
            ):
                for mt in range(MTILES):
                    row0 = mt * P
                    xq = xsign.tile([P, K], BF16, tag="xq")
                    for h in range(NCH):
                        xt = xload.tile([P, XCH], F32, tag="xt")
                        nc.sync.dma_start(xt, x_in[ds(row0, P), ts(h, XCH)])
                        nc.scalar.activation(
                            xq[:, ts(h, XCH)], xt,
                            mybir.ActivationFunctionType.Sign,
                        )
                    xqTb = xtr.tile([P, KSUB, P], BF16, tag="xqTb")
                    for ks in range(KSUB):
                        nc.sync.dma_start(
                            xqTb[:, ks, :], xq[:, ts(ks, P)], transpose=True
                        )
                    xqT8 = xq8p.tile([P, KSUB, P], FP8, tag="xqT8")
                    nc.vector.tensor_copy(xqT8, xqTb)

                    pst = [
                        psp.tile([P, N_FREE], F32, tag=f"ps{n}", name=f"ps{n}")
                        for n in range(NT)
                    ]
                    for kp in range(KSUB // 2):
                        lhs = xqT8[:, 2 * kp : 2 * kp + 2, :]
                        for n in range(NT):
                            nc.tensor.matmul(
                                pst[n],
                                lhs,
                                wqT8[:, 2 * kp : 2 * kp + 2, ts(n, N_FREE)],
                                start=(kp == 0),
                                stop=(kp == KSUB // 2 - 1),
                                perf_mode=mybir.MatmulPerfMode.DoubleRow,
                            )
                    for n in range(NT):
                        ot = oev.tile([P, N_FREE], F32, tag=f"ot{n}", name=f"ot{n}")
                        nc.vector.scalar_tensor_tensor(
                            ot,
                            pst[n],
                            th_pos[:, 0:1],
                            bias_bc[:, ts(n, N_FREE)],
                            mybir.AluOpType.mult,
                            mybir.AluOpType.add,
                        )
                        nc.sync.dma_start(
                            out_d[ds(row0, P), ts(n, N_FREE)], ot
                        )

    nc.compile()
    return nc


_CACHE = {}

BUILDERS = {"v1": build_nc, "v2": build_nc_v2}


def _get_nc():
    ver = os.environ.get("BITNET_VERSION", "v2")
    key = f"nc_{ver}"
    if key not in _CACHE:
        _CACHE[key] = BUILDERS[ver]()
    return _CACHE[key]


def kernel(x, weight, bias):
    x = np.ascontiguousarray(np.asarray(x, dtype=np.float32))
    weight = np.ascontiguousarray(np.asarray(weight, dtype=np.float32))
    bias = np.ascontiguousarray(np.asarray(bias, dtype=np.float32))

    xf = x.reshape(B * S, DIN)
    nshard = DOUT // N_CORES
    nc = _get_nc()

    in_maps = []
    for c in range(N_CORES):
        in_maps.append(
            {
                "x": xf,
                "w": weight[c * nshard : (c + 1) * nshard],
                "b": bias[c * nshard : (c + 1) * nshard],
            }
        )

    res = run_bass_kernel_spmd(
        nc,
        in_maps,
        core_ids=list(range(N_CORES)),
        trace=bool(int(os.environ.get("BITNET_TRACE", "0"))),
    )
    _CACHE["last_result"] = res
    shards = [r["out"] for r in res.results]
    out = np.concatenate(shards, axis=1)  # [M, DOUT]
    return out.reshape(B, S, DOUT)
